# revision 1
# baseline (speedup 1.0000x reference)
"""DecoderLSTM Trainium2 kernel.

Data-parallel over batch: B=512 is sharded 64-per-core across 8 NeuronCores;
LSTM/FC weights are replicated and streamed from HBM each step (63 MB fp32
exceeds the 24 MB SBUF).  All matmuls run in fp32: the 96-step recurrence is
chaotic (measured error amplification ~250x), so reduced-precision matmuls
diverge (bf16 -> 60% rel err, fp32r -> 2.4%); fp32 lands ~1e-4.

Per-core layout:
  Big matmuls ("orientation A"): out[b, feat] accumulates in PSUM, lhsT =
  transposed activations [K, 64] stationary, rhs = streamed weight tiles
  [K, 512] moving.  Gate biases are added during PSUM evacuation on DVE.
  Small matmuls (embedding, fc2) run "orientation B" (weights stationary),
  producing transposed outputs directly — y feeds back as x with no
  transpose.  h0/h1/relu transposes use the PE transpose path.

Self-contained: shapes hardcoded; nothing read from the problem directory.
"""
from contextlib import ExitStack

import numpy as np

import concourse.bass as bass
import concourse.tile as tile
from concourse import bacc, mybir
from concourse import bass_utils

F32 = mybir.dt.float32
AF = mybir.ActivationFunctionType
ALU = mybir.AluOpType

B, D, E, H, T_FULL = 512, 64, 512, 1024, 96
NC = 8
BC = B // NC          # 64 batch rows per core
KC_E = E // 128       # 4
KC_H = H // 128       # 8
G4H = 4 * H           # 4096
LN_EPS = 1e-5

_cache = {}


def _emit(ctx: ExitStack, tc: tile.TileContext, io: dict, t_steps: int):
    nc = tc.nc

    res = ctx.enter_context(tc.tile_pool(name="resident", bufs=1))
    state = ctx.enter_context(tc.tile_pool(name="state", bufs=2))
    work = ctx.enter_context(tc.tile_pool(name="work", bufs=1))
    wstream = ctx.enter_context(tc.tile_pool(name="wstream", bufs=3))
    gpsum = ctx.enter_context(tc.tile_pool(name="gpsum", bufs=4, space="PSUM"))
    tpsum = ctx.enter_context(tc.tile_pool(name="tpsum", bufs=2, space="PSUM"))
    spsum = ctx.enter_context(tc.tile_pool(name="spsum", bufs=2, space="PSUM"))

    # ---- resident tensors (loaded once) ----
    emb_W = res.tile([64, E], F32)               # [D, E]; lhsT chunks [:, c*128:]
    fc2_W = res.tile([128, KC_H, 64], F32)       # fc2 lhsT chunks
    ident = res.tile([64, 64], F32)
    b0_bc = res.tile([BC, G4H], F32)             # gate biases bcast over batch
    b1_bc = res.tile([BC, G4H], F32)
    fc1_b_bc = res.tile([BC, H], F32)
    emb_bT = res.tile([128, KC_E], F32)          # per-partition bias, chunk c
    fc2_bT = res.tile([64, 1], F32)
    g_bc = res.tile([BC, H], F32)
    bb_bc = res.tile([BC, H], F32)

    for name, t in [("emb_W", emb_W), ("ident", ident), ("b0_bc", b0_bc),
                    ("b1_bc", b1_bc), ("fc1_b_bc", fc1_b_bc),
                    ("emb_bT", emb_bT), ("fc2_bT", fc2_bT), ("g_bc", g_bc),
                    ("bb_bc", bb_bc)]:
        nc.sync.dma_start(t[:], io[name].ap())
    nc.sync.dma_start(fc2_W[:], io["fc2_W"].ap().rearrange("(k p) o -> p k o", p=128))

    # ---- initial state ----
    xT0 = res.tile([64, BC], F32)
    nc.sync.dma_start(xT0[:], io["xT0"].ap())
    eps_t = res.tile([BC, 1], F32)
    nc.vector.memset(eps_t[:], LN_EPS)
    h0T = state.tile([128, KC_H, BC], F32, tag="h0T")
    h1T = state.tile([128, KC_H, BC], F32, tag="h1T")
    c0 = state.tile([BC, H], F32, tag="c0")
    c1 = state.tile([BC, H], F32, tag="c1")
    nc.sync.dma_start(h0T[:], io["h0T0"].ap().rearrange("(k p) b -> p k b", p=128))
    nc.sync.dma_start(h1T[:], io["h1T0"].ap().rearrange("(k p) b -> p k b", p=128))
    nc.sync.dma_start(c0[:], io["c00"].ap())
    nc.sync.dma_start(c1[:], io["c10"].ap())

    y_last = None

    def lstm_layer(layer, xe_lhsT, hT_prev, c_prev, w_in_dram, w_hh_dram,
                   b_bc, kc_in):
        """Gates + cell update.  Returns (h_new [BC,H] sbuf, c_new)."""
        gts = work.tile([BC, G4H], F32, tag=f"gts{layer}")
        for half in range(2):
            gb = [gpsum.tile([BC, 512], F32, tag="gb", name=f"gb{half}_{_n}")
                  for _n in range(4)]
            # recurrent part first (hT_prev ready since last step)
            for k in range(KC_H):
                wt = wstream.tile([128, 2048], F32, tag="wstream")
                nc.sync.dma_start(
                    wt[:], w_hh_dram.ap()[k * 128:(k + 1) * 128,
                                          half * 2048:(half + 1) * 2048])
                for n in range(4):
                    nc.tensor.matmul(gb[n][:], hT_prev[:, k, :],
                                     wt[:, n * 512:(n + 1) * 512],
                                     start=(k == 0), stop=False)
            # input part
            for k in range(kc_in):
                wt = wstream.tile([128, 2048], F32, tag="wstream")
                nc.sync.dma_start(
                    wt[:], w_in_dram.ap()[k * 128:(k + 1) * 128,
                                          half * 2048:(half + 1) * 2048])
                lhsT = xe_lhsT(k)
                for n in range(4):
                    nc.tensor.matmul(gb[n][:], lhsT,
                                     wt[:, n * 512:(n + 1) * 512],
                                     start=False, stop=(k == kc_in - 1))
            # evacuate with bias add (DVE), then in-place nonlinearity (ACT)
            for n in range(4):
                col = half * 2048 + n * 512
                nc.vector.tensor_add(gts[:, col:col + 512], gb[n][:],
                                     b_bc[:, col:col + 512])
        # i f g o, each H wide
        nc.scalar.activation(gts[:, 0:2 * H], gts[:, 0:2 * H], AF.Sigmoid)
        nc.scalar.activation(gts[:, 2 * H:3 * H], gts[:, 2 * H:3 * H], AF.Tanh)
        nc.scalar.activation(gts[:, 3 * H:], gts[:, 3 * H:], AF.Sigmoid)

        c_new = state.tile([BC, H], F32, tag=f"c{layer}")
        tmp1 = work.tile([BC, H], F32, tag="tmp1")
        tanh_c = work.tile([BC, H], F32, tag=f"tanh_c{layer}")
        h_new = work.tile([BC, H], F32, tag=f"h{layer}")
        nc.vector.tensor_mul(tmp1[:], gts[:, H:2 * H], c_prev[:])
        nc.vector.tensor_mul(c_new[:], gts[:, 0:H], gts[:, 2 * H:3 * H])
        nc.vector.tensor_add(c_new[:], c_new[:], tmp1[:])
        nc.scalar.activation(tanh_c[:], c_new[:], AF.Tanh)
        nc.vector.tensor_mul(h_new[:], gts[:, 3 * H:], tanh_c[:])
        return h_new, c_new

    def transpose_to(hT_new, h_sb):
        """h [BC, H] -> hT [128, KC_H, BC] via PE transposes."""
        for ck in range(KC_H):
            tp = tpsum.tile([128, BC], F32, tag="tp")
            nc.tensor.transpose(tp[:], h_sb[:, ck * 128:(ck + 1) * 128],
                                ident[:])
            nc.vector.tensor_copy(hT_new[:, ck, :], tp[:])

    for t in range(t_steps):
        xT = xT0[:] if t == 0 else y_last[:]

        # ---- embedding (orientation B): xeT[c] = emb_W[:,c].T @ xT ----
        xeT = work.tile([128, KC_E, BC], F32, tag="xeT")
        for c in range(KC_E):
            xp = spsum.tile([128, BC], F32, tag="sp")
            nc.tensor.matmul(xp[:], emb_W[:, c * 128:(c + 1) * 128], xT,
                             start=True, stop=True)
            nc.vector.tensor_scalar_add(xeT[:, c, :], xp[:], emb_bT[:, c:c + 1])

        # ---- LSTM layers ----
        h0_new, c0_new = lstm_layer(
            0, lambda k: xeT[:, k, :], h0T, c0,
            io["W_ih0"], io["W_hh0"], b0_bc, KC_E)
        h0T_new = state.tile([128, KC_H, BC], F32, tag="h0T")
        transpose_to(h0T_new, h0_new)

        h1_new, c1_new = lstm_layer(
            1, lambda k: h0T_new[:, k, :], h1T, c1,
            io["W_ih1"], io["W_hh1"], b1_bc, KC_H)
        h1T_new = state.tile([128, KC_H, BC], F32, tag="h1T")
        transpose_to(h1T_new, h1_new)

        # ---- fc1 + LayerNorm + ReLU ----
        z = work.tile([BC, H], F32, tag="z")
        z_sums = work.tile([BC, 2], F32, tag="z_sums")
        zp = [spsum.tile([BC, 512], F32, tag="sp", name=f"zp{_n}")
               for _n in range(2)]
        for k in range(KC_H):
            wt = wstream.tile([128, H], F32, tag="wstream")
            nc.sync.dma_start(wt[:],
                              io["fc1_W"].ap()[k * 128:(k + 1) * 128, :])
            for n in range(2):
                nc.tensor.matmul(zp[n][:], h1T_new[:, k, :],
                                 wt[:, n * 512:(n + 1) * 512],
                                 start=(k == 0), stop=(k == KC_H - 1))
        for n in range(2):
            nc.vector.tensor_add(z[:, n * 512:(n + 1) * 512], zp[n][:],
                                 fc1_b_bc[:, n * 512:(n + 1) * 512])
            nc.vector.reduce_sum(z_sums[:, n:n + 1],
                                 z[:, n * 512:(n + 1) * 512],
                                 axis=mybir.AxisListType.X)
        mu = work.tile([BC, 1], F32, tag="mu")
        negmu = work.tile([BC, 1], F32, tag="negmu")
        sqs = work.tile([BC, 1], F32, tag="sqs")
        sq = work.tile([BC, H], F32, tag="sq")
        va = work.tile([BC, 1], F32, tag="va")
        sv = work.tile([BC, 1], F32, tag="sv")
        rstd = work.tile([BC, 1], F32, tag="rstd")
        nc.vector.tensor_add(mu[:], z_sums[:, 0:1], z_sums[:, 1:2])
        nc.vector.tensor_scalar_mul(negmu[:], mu[:], -1.0 / H)
        nc.vector.tensor_scalar_mul(mu[:], mu[:], 1.0 / H)
        nc.scalar.activation(sq[:], z[:], AF.Square, bias=negmu[:],
                             accum_out=sqs[:])
        nc.vector.tensor_scalar_mul(va[:], sqs[:], 1.0 / H)
        nc.scalar.activation(sv[:], va[:], AF.Sqrt, bias=eps_t[:])
        nc.vector.reciprocal(rstd[:], sv[:])
        zn = work.tile([BC, H], F32, tag="zn")
        nc.vector.tensor_scalar(zn[:], z[:], mu[:], rstd[:],
                                ALU.subtract, ALU.mult)
        nc.vector.tensor_mul(zn[:], zn[:], g_bc[:])
        nc.vector.tensor_add(zn[:], zn[:], bb_bc[:])
        nc.scalar.activation(zn[:], zn[:], AF.Relu)

        reluT = work.tile([128, KC_H, BC], F32, tag="reluT")
        transpose_to(reluT, zn)

        # ---- fc2 (orientation B): yT = fc2_W.T @ reluT ----
        yp = spsum.tile([64, BC], F32, tag="sp")
        for k in range(KC_H):
            nc.tensor.matmul(yp[:], fc2_W[:, k, :], reluT[:, k, :],
                             start=(k == 0), stop=(k == KC_H - 1))
        y_new = state.tile([64, BC], F32, tag="ylast")
        nc.vector.tensor_scalar_add(y_new[:], yp[:], fc2_bT[:])
        nc.sync.dma_start(io["ysT"].ap()[:, t, :], y_new[:])

        h0T, h1T, c0, c1, y_last = h0T_new, h1T_new, c0_new, c1_new, y_new


def build(t_steps=T_FULL):
    if t_steps in _cache:
        return _cache[t_steps]
    nc = bacc.Bacc("TRN2", target_bir_lowering=False, debug=False)
    io = {}
    inputs = [
        ("xT0", (64, BC)), ("h0T0", (H, BC)), ("h1T0", (H, BC)),
        ("c00", (BC, H)), ("c10", (BC, H)),
        ("W_ih0", (E, G4H)), ("W_hh0", (H, G4H)),
        ("W_ih1", (H, G4H)), ("W_hh1", (H, G4H)),
        ("fc1_W", (H, H)), ("fc2_W", (H, 64)), ("emb_W", (64, E)),
        ("b0_bc", (BC, G4H)), ("b1_bc", (BC, G4H)),
        ("fc1_b_bc", (BC, H)), ("emb_bT", (128, KC_E)),
        ("fc2_bT", (64, 1)), ("g_bc", (BC, H)), ("bb_bc", (BC, H)),
        ("ident", (64, 64)),
    ]
    for name, shape in inputs:
        io[name] = nc.dram_tensor(name, shape, F32, kind="ExternalInput")
    io["ysT"] = nc.dram_tensor("ysT", (64, t_steps, BC), F32,
                               kind="ExternalOutput")
    with tile.TileContext(nc) as tc:
        with ExitStack() as ctx:
            _emit(ctx, tc, io, t_steps)
    nc.compile()
    _cache[t_steps] = (nc, io)
    return nc, io


def make_in_maps(inputs):
    """Shard + transform full inputs into 8 per-core input maps."""
    f = lambda x: np.ascontiguousarray(np.asarray(x), dtype=np.float32)
    x0 = f(inputs["x_0"])
    hn = f(inputs["h_n"])
    cn = f(inputs["c_n"])
    base = {
        "W_ih0": f(inputs["W_ih0"]), "W_hh0": f(inputs["W_hh0"]),
        "W_ih1": f(inputs["W_ih1"]), "W_hh1": f(inputs["W_hh1"]),
        "fc1_W": f(inputs["fc1_W"]), "fc2_W": f(inputs["fc2_W"]),
        "emb_W": f(inputs["emb_W"]),
        "b0_bc": np.tile((f(inputs["b_ih0"]) + f(inputs["b_hh0"]))[None, :],
                         (BC, 1)),
        "b1_bc": np.tile((f(inputs["b_ih1"]) + f(inputs["b_hh1"]))[None, :],
                         (BC, 1)),
        "fc1_b_bc": np.tile(f(inputs["fc1_b"])[None, :], (BC, 1)),
        "emb_bT": np.ascontiguousarray(f(inputs["emb_b"]).reshape(KC_E, 128).T),
        "fc2_bT": f(inputs["fc2_b"])[:, None],
        "g_bc": np.tile(f(inputs["ln_g"])[None, :], (BC, 1)),
        "bb_bc": np.tile(f(inputs["ln_b"])[None, :], (BC, 1)),
        "ident": np.eye(64, dtype=np.float32),
    }
    in_maps = []
    for c in range(NC):
        sl = slice(c * BC, (c + 1) * BC)
        m = dict(base)
        m["xT0"] = np.ascontiguousarray(x0[sl].T)
        m["h0T0"] = np.ascontiguousarray(hn[0, sl].T)
        m["h1T0"] = np.ascontiguousarray(hn[1, sl].T)
        m["c00"] = np.ascontiguousarray(cn[0, sl])
        m["c10"] = np.ascontiguousarray(cn[1, sl])
        in_maps.append(m)
    return in_maps


def kernel(**inputs):
    t_steps = int(inputs.get("forecast_window", T_FULL))
    nc, io = build(t_steps)
    in_maps = make_in_maps(inputs)
    r = bass_utils.run_bass_kernel_spmd(nc, in_maps, core_ids=list(range(NC)))
    out = np.empty((B, t_steps, D), np.float32)
    for c in range(NC):
        ysT = r.results[c]["ysT"]              # [D, t, BC]
        out[c * BC:(c + 1) * BC] = ysT.transpose(2, 1, 0)
    return out



# revision 2
# speedup vs baseline: 1.0404x; 1.0404x over previous
"""DecoderLSTM Trainium2 kernel v2: weight-resident tensor-parallel.

Topology: 8 NeuronCores = 2 batch groups x 4 feature shards.
Each group of 4 cores handles 256 batch rows; within a group, LSTM gate
columns are sharded 4-way (256 h-units per core) so all weights stay
resident in SBUF (no per-step weight streaming, which bounded the
data-parallel baseline at ~16 ms of DMA).  Per step each group runs two
4-core AllGathers (h0, h1, as bf16 hi/lo pairs).  The fc head
(fc1+LN+ReLU+fc2) and the embedding are replicated within the group so
no further collectives are needed.

All matmuls use split-bf16 3-pass (hi/lo decomposition: HH + LH + HL),
~2^-16 effective input precision at 1 cycle/row on the PE vs fp32's 4
cycles/row.  Cell state c stays fp32 and sharded.  LayerNorm stats are
fp32 on DVE (batch-major fc1) - per-step LN noise would otherwise be
amplified by the chaotic 96-step recurrence.

PSUM choreography (8 banks): gates0 accumulates in banks 0-3 across the
AG(h1) window; gates1 / fc1 / transposes / fc2 / xe rotate through banks
4-7 via one shared pool tag (their lifetimes are disjoint).

Self-contained: shapes hardcoded; nothing read from the problem dir.
"""
from contextlib import ExitStack

import numpy as np
import ml_dtypes

import concourse.bass as bass
import concourse.tile as tile
from concourse import bacc, mybir
from concourse import bass_utils

F32 = mybir.dt.float32
BF16 = mybir.dt.bfloat16
AF = mybir.ActivationFunctionType
ALU = mybir.AluOpType
BFNP = ml_dtypes.bfloat16

B, D, E, H, T_FULL = 512, 64, 512, 1024, 96
NC = 8
NG = 2                 # batch groups
NF = 4                 # feature shards per group
BG = B // NG           # 256 batch rows per group
HS = H // NF           # 256 h-units per core
KC_E = E // 128        # 4
KC_H = H // 128        # 8
LN_EPS = 1e-5
GROUPS = [[0, 1, 2, 3], [4, 5, 6, 7]]

_cache = {}


def _emit(ctx: ExitStack, tc: tile.TileContext, io: dict, t_steps: int):
    nc = tc.nc

    res = ctx.enter_context(tc.tile_pool(name="res", bufs=1))
    work = ctx.enter_context(tc.tile_pool(name="work", bufs=1))
    dram = ctx.enter_context(tc.tile_pool(name="dram", bufs=2, space="DRAM"))
    gp0 = ctx.enter_context(tc.tile_pool(name="gp0", bufs=1, space="PSUM"))
    gp1 = ctx.enter_context(tc.tile_pool(name="gp1", bufs=1, space="PSUM"))

    # ---- resident weights / constants / state (loaded once) ----
    w = {}
    for name, shape, dt in _IO_SPECS:
        if name.startswith("_out"):
            continue
        w[name] = res.tile(list(shape), dt, name=name)
        nc.sync.dma_start(w[name][:], io[name].ap())

    eps_t = res.tile([128, 1], F32)
    nc.vector.memset(eps_t[:], LN_EPS)

    h0T_hi, h0T_lo = w["h0T0_hi"], w["h0T0_lo"]
    h1T_hi, h1T_lo = w["h1T0_hi"], w["h1T0_lo"]
    yT_hi, yT_lo = w["xT0_hi"], w["xT0_lo"]
    c0, c1 = w["c00"], w["c10"]

    xeT_hi = res.tile([128, KC_E, BG], BF16)
    xeT_lo = res.tile([128, KC_E, BG], BF16)
    rT_hi = res.tile([128, KC_H, BG], BF16)   # relu out, feature-major
    rT_lo = res.tile([128, KC_H, BG], BF16)

    def mm3(out, wh, wl, rh, rl, start, stop):
        nc.tensor.matmul(out, wh, rh, start=start, stop=False)
        nc.tensor.matmul(out, wl, rh, start=False, stop=False)
        nc.tensor.matmul(out, wh, rl, start=False, stop=stop)

    def gate_mm(G, Wh, Wl, act_hi, act_lo, ks, start, stop):
        """Accumulate sum_k W[k].T @ act[k] into gate psum G [128, 2048]."""
        for i, k in enumerate(ks):
            rh = act_hi(k)
            rl = act_lo(k)
            for ct in range(8):
                # one accumulation group per 2KB psum bank: start zeroes the
                # whole bank, so only ct-even opens and ct-odd closes it
                mm3(G[:, ct * 256:(ct + 1) * 256],
                    Wh[:, k, ct * 128:(ct + 1) * 128],
                    Wl[:, k, ct * 128:(ct + 1) * 128],
                    rh, rl,
                    start and i == 0 and ct % 2 == 0,
                    stop and i == len(ks) - 1 and ct % 2 == 1)

    def cell(G, bT, c, layer):
        """Nonlinearities (in psum, with per-partition bias) + cell update.
        Gate layout: col-tiles [i0 i1 f0 f1 g0 g1 o0 o1].  Returns h hi/lo."""
        for ct in [0, 1, 2, 3, 6, 7]:   # sigmoids in place (psum)
            reg = G[:, ct * 256:(ct + 1) * 256]
            nc.scalar.activation(reg, reg, AF.Sigmoid, bias=bT[:, ct:ct + 1])
        # tanh(g) lands in SBUF: DVE can read only one psum operand per op
        tg = work.tile([128, 512], F32, tag="tnc", name=f"tg_{layer}")
        for s in range(2):
            ct = 4 + s
            nc.scalar.activation(tg[:, s * 256:(s + 1) * 256],
                                 G[:, ct * 256:(ct + 1) * 256],
                                 AF.Tanh, bias=bT[:, ct:ct + 1])
        i_, f_, o_ = G[:, 0:512], G[:, 512:1024], G[:, 1536:2048]
        t1 = work.tile([128, 512], F32, tag="t1", name=f"t1_{layer}")
        nc.vector.tensor_mul(t1[:], f_, c[:])
        nc.vector.tensor_mul(c[:], i_, tg[:])
        nc.vector.tensor_add(c[:], c[:], t1[:])
        tnc = work.tile([128, 512], F32, tag="tnc", name=f"tnc_{layer}")
        nc.scalar.activation(tnc[:], c[:], AF.Tanh)
        hf = work.tile([128, 512], F32, tag="t1", name=f"hf_{layer}")
        nc.vector.tensor_mul(hf[:], o_, tnc[:])
        hh = work.tile([128, 512], BF16, tag="hh", bufs=1, name=f"hh_{layer}")
        hl = work.tile([128, 512], BF16, tag="hl", bufs=1, name=f"hl_{layer}")
        nc.scalar.activation(hh[:], hf[:], AF.Copy)
        nc.vector.tensor_sub(hl[:], hf[:], hh[:])
        return hh, hl

    def allgather_h(hh, hl, dst_hi, dst_lo, tagn):
        """4-core AG of own h slice (hi/lo bf16) -> full [H, BG] hi/lo."""
        bin_ = dram.tile([2, 128, 2, BG], BF16, tag="hbin", name=f"bin{tagn}")
        bout = dram.tile([NF, 2, 128, 2, BG], BF16, tag="hbout",
                         name=f"bout{tagn}")
        nc.sync.dma_start(bin_[0], hh[:].rearrange("p (k b) -> p k b", k=2))
        nc.sync.dma_start(bin_[1], hl[:].rearrange("p (k b) -> p k b", k=2))
        nc.gpsimd.collective_compute(
            "AllGather", ALU.bypass, replica_groups=GROUPS,
            ins=[bin_[:].opt()], outs=[bout[:].opt()])
        src = bout[:].rearrange("r l p k b -> l p r k b")
        nc.sync.dma_start(dst_hi[:].rearrange("p (r k) b -> p r k b", r=NF),
                          src[0])
        nc.sync.dma_start(dst_lo[:].rearrange("p (r k) b -> p r k b", r=NF),
                          src[1])

    def fc_head(t):
        """fc1 (batch-major) + LN + ReLU + transpose + fc2 + y out + hi/lo."""
        for bt in range(2):
            F = gp1.tile([128, 1024], F32, tag="G1F", name=f"fps{bt}_{t}")
            for n in range(2):
                out = F[:, n * 512:(n + 1) * 512]
                nc.tensor.matmul(out, w["ones1"][:],
                                 w["fc1b_hi"][:, n * 512:(n + 1) * 512],
                                 start=True, stop=False)
                nc.tensor.matmul(out, w["ones1"][:],
                                 w["fc1b_lo"][:, n * 512:(n + 1) * 512],
                                 start=False, stop=False)
                for k in range(KC_H):
                    lh = h1T_hi[:, k, bt * 128:(bt + 1) * 128]
                    ll = h1T_lo[:, k, bt * 128:(bt + 1) * 128]
                    wh = w["fc1W_hi"][:, k, n * 512:(n + 1) * 512]
                    wl = w["fc1W_lo"][:, k, n * 512:(n + 1) * 512]
                    nc.tensor.matmul(out, lh, wh, start=False, stop=False)
                    nc.tensor.matmul(out, ll, wh, start=False, stop=False)
                    nc.tensor.matmul(out, lh, wl, start=False,
                                     stop=(k == KC_H - 1))
            # LayerNorm over the free axis, fp32 (stat noise would be
            # amplified ~250x by the recurrence if done in low precision)
            zs = work.tile([128, 1], F32, tag="zs", name=f"zs{bt}")
            nc.vector.reduce_sum(zs[:], F[:], axis=mybir.AxisListType.X)
            negmu = work.tile([128, 1], F32, tag="negmu", name=f"nmu{bt}")
            mu = work.tile([128, 1], F32, tag="mu", name=f"mu{bt}")
            nc.vector.tensor_scalar_mul(negmu[:], zs[:], -1.0 / H)
            nc.vector.tensor_scalar_mul(mu[:], zs[:], 1.0 / H)
            zscr = work.tile([128, 1024], F32, tag="zscr", bufs=1,
                             name=f"zscr{bt}")
            sqs = work.tile([128, 1], F32, tag="sqs", name=f"sqs{bt}")
            nc.scalar.activation(zscr[:], F[:], AF.Square, bias=negmu[:],
                                 accum_out=sqs[:])
            va = work.tile([128, 1], F32, tag="va", name=f"va{bt}")
            sv = work.tile([128, 1], F32, tag="sv", name=f"sv{bt}")
            rstd = work.tile([128, 1], F32, tag="rstd", name=f"rstd{bt}")
            nc.vector.tensor_scalar_mul(va[:], sqs[:], 1.0 / H)
            nc.scalar.activation(sv[:], va[:], AF.Sqrt, bias=eps_t[:])
            nc.vector.reciprocal(rstd[:], sv[:])
            nc.vector.tensor_scalar(zscr[:], F[:], mu[:], rstd[:],
                                    ALU.subtract, ALU.mult)
            nc.vector.tensor_mul(zscr[:], zscr[:], w["g_bc"][:])
            nc.vector.tensor_add(zscr[:], zscr[:], w["bb_bc"][:])
            nc.scalar.activation(zscr[:], zscr[:], AF.Relu)
            # fp32 transposes to feature-major; evac casts hi/lo
            TP = gp1.tile([128, KC_H, 128], F32, tag="G1F",
                          name=f"tp{bt}_{t}")
            for ct in range(KC_H):
                # 4 fp32 transposes share each 2KB bank: group start/stop
                nc.tensor.matmul(TP[:, ct, :],
                                 zscr[:, ct * 128:(ct + 1) * 128],
                                 w["ident"][:], is_transpose=True,
                                 start=(ct % 4 == 0), stop=(ct % 4 == 3))
            bsl = slice(bt * 128, (bt + 1) * 128)
            nc.scalar.activation(rT_hi[:, :, bsl], TP[:], AF.Copy)
            nc.vector.tensor_sub(rT_lo[:, :, bsl], TP[:], rT_hi[:, :, bsl])
        # fc2 (feature-major): yT = sum_k fc2W[k].T @ reluT[k]
        yp = gp1.tile([64, BG], F32, tag="G1F", name=f"yps{t}")
        for k in range(KC_H):
            mm3(yp[:], w["fc2W_hi"][:, k, :], w["fc2W_lo"][:, k, :],
                rT_hi[:, k, :], rT_lo[:, k, :], k == 0, k == KC_H - 1)
        ysb = work.tile([64, BG], F32, tag="ysb", name=f"ysb{t}")
        nc.vector.tensor_scalar_add(ysb[:], yp[:], w["fc2bT"][:])
        nc.sync.dma_start(io["ysT"].ap()[:, t, :], ysb[:])
        nc.scalar.activation(yT_hi[:], ysb[:], AF.Copy)
        nc.vector.tensor_sub(yT_lo[:], ysb[:], yT_hi[:])

    def xe_block(t):
        """Embedding: xeT[et] = embW[:, et].T @ yT (+ bias), hi/lo out."""
        for et in range(KC_E):
            xp = gp1.tile([128, BG], F32, tag="G1F", name=f"xeps{et}_{t}")
            mm3(xp[:], w["embW_hi"][:, et * 128:(et + 1) * 128],
                w["embW_lo"][:, et * 128:(et + 1) * 128],
                yT_hi[:], yT_lo[:], True, True)
            xef = work.tile([128, BG], F32, tag="xef", bufs=2,
                            name=f"xef{et}_{t}")
            nc.vector.tensor_scalar_add(xef[:], xp[:],
                                        w["embbT"][:, et:et + 1])
            nc.scalar.activation(xeT_hi[:, et, :], xef[:], AF.Copy)
            nc.vector.tensor_sub(xeT_lo[:, et, :], xef[:], xeT_hi[:, et, :])

    # ---- prologue ----
    xe_block("p")
    G0 = gp0.tile([128, 2048], F32, tag="G0", name="G0p")
    gate_mm(G0, w["Whh0_hi"], w["Whh0_lo"],
            lambda k: h0T_hi[:, k, :], lambda k: h0T_lo[:, k, :],
            range(KC_H), start=True, stop=False)

    # ---- time loop ----
    for t in range(t_steps):
        gate_mm(G0, w["Wih0_hi"], w["Wih0_lo"],
                lambda k: xeT_hi[:, k, :], lambda k: xeT_lo[:, k, :],
                range(KC_E), start=False, stop=True)
        h0h, h0l = cell(G0, w["b0T"], c0, 0)
        allgather_h(h0h, h0l, h0T_hi, h0T_lo, f"a{t}")

        G1 = gp1.tile([128, 2048], F32, tag="G1F", name=f"G1_{t}")
        gate_mm(G1, w["Whh1_hi"], w["Whh1_lo"],
                lambda k: h1T_hi[:, k, :], lambda k: h1T_lo[:, k, :],
                range(KC_H), start=True, stop=False)
        gate_mm(G1, w["Wih1_hi"], w["Wih1_lo"],
                lambda k: h0T_hi[:, k, :], lambda k: h0T_lo[:, k, :],
                range(KC_H), start=False, stop=True)
        h1h, h1l = cell(G1, w["b1T"], c1, 1)
        allgather_h(h1h, h1l, h1T_hi, h1T_lo, f"b{t}")

        if t + 1 < t_steps:
            G0 = gp0.tile([128, 2048], F32, tag="G0", name=f"G0_{t}")
            gate_mm(G0, w["Whh0_hi"], w["Whh0_lo"],
                    lambda k: h0T_hi[:, k, :], lambda k: h0T_lo[:, k, :],
                    range(KC_H), start=True, stop=False)

        fc_head(t)
        if t + 1 < t_steps:
            xe_block(t)


_IO_SPECS = [
    ("Wih0_hi", (128, KC_E, 1024), BF16), ("Wih0_lo", (128, KC_E, 1024), BF16),
    ("Whh0_hi", (128, KC_H, 1024), BF16), ("Whh0_lo", (128, KC_H, 1024), BF16),
    ("Wih1_hi", (128, KC_H, 1024), BF16), ("Wih1_lo", (128, KC_H, 1024), BF16),
    ("Whh1_hi", (128, KC_H, 1024), BF16), ("Whh1_lo", (128, KC_H, 1024), BF16),
    ("b0T", (128, 8), F32), ("b1T", (128, 8), F32),
    ("fc1W_hi", (128, KC_H, 1024), BF16), ("fc1W_lo", (128, KC_H, 1024), BF16),
    ("fc1b_hi", (1, 1024), BF16), ("fc1b_lo", (1, 1024), BF16),
    ("ones1", (1, 128), BF16),
    ("g_bc", (128, 1024), F32), ("bb_bc", (128, 1024), F32),
    ("fc2W_hi", (128, KC_H, 64), BF16), ("fc2W_lo", (128, KC_H, 64), BF16),
    ("fc2bT", (64, 1), F32),
    ("embW_hi", (64, 512), BF16), ("embW_lo", (64, 512), BF16),
    ("embbT", (128, KC_E), F32),
    ("ident", (128, 128), F32),
    ("xT0_hi", (64, BG), BF16), ("xT0_lo", (64, BG), BF16),
    ("h0T0_hi", (128, KC_H, BG), BF16), ("h0T0_lo", (128, KC_H, BG), BF16),
    ("h1T0_hi", (128, KC_H, BG), BF16), ("h1T0_lo", (128, KC_H, BG), BF16),
    ("c00", (128, 2 * BG), F32), ("c10", (128, 2 * BG), F32),
]


def build(t_steps=T_FULL):
    if t_steps in _cache:
        return _cache[t_steps]
    nc = bacc.Bacc("TRN2", target_bir_lowering=False, debug=False,
                   num_devices=NC)
    io = {}
    for name, shape, dt in _IO_SPECS:
        io[name] = nc.dram_tensor(name, shape, dt, kind="ExternalInput")
    io["ysT"] = nc.dram_tensor("ysT", (64, t_steps, BG), F32,
                               kind="ExternalOutput")
    with tile.TileContext(nc) as tc:
        with ExitStack() as ctx:
            _emit(ctx, tc, io, t_steps)
    nc.compile()
    _cache[t_steps] = (nc, io)
    return nc, io


def _hilo(x):
    hi = x.astype(BFNP)
    lo = (x - hi.astype(np.float32)).astype(BFNP)
    return np.ascontiguousarray(hi), np.ascontiguousarray(lo)


def make_in_maps(inputs):
    f = lambda x: np.asarray(x, dtype=np.float32)
    x0, hn, cn = f(inputs["x_0"]), f(inputs["h_n"]), f(inputs["c_n"])
    Wg = {l: {"ih": f(inputs[f"W_ih{l}"]), "hh": f(inputs[f"W_hh{l}"]),
              "b": f(inputs[f"b_ih{l}"]) + f(inputs[f"b_hh{l}"])}
          for l in (0, 1)}

    def kmaj(x):  # [K, C] -> [128, K/128, C]
        K, C = x.shape
        return np.ascontiguousarray(
            x.reshape(K // 128, 128, C).transpose(1, 0, 2))

    fc1_hi, fc1_lo = _hilo(kmaj(f(inputs["fc1_W"])))
    fc2_hi, fc2_lo = _hilo(kmaj(f(inputs["fc2_W"])))
    fc1b_hi, fc1b_lo = _hilo(f(inputs["fc1_b"])[None, :])
    emb_hi, emb_lo = _hilo(f(inputs["emb_W"]))
    emb_b = f(inputs["emb_b"])

    base = {
        "fc1W_hi": fc1_hi, "fc1W_lo": fc1_lo,
        "fc1b_hi": fc1b_hi, "fc1b_lo": fc1b_lo,
        "ones1": np.ones((1, 128), BFNP),
        "g_bc": np.ascontiguousarray(np.tile(f(inputs["ln_g"])[None, :],
                                             (128, 1))),
        "bb_bc": np.ascontiguousarray(np.tile(f(inputs["ln_b"])[None, :],
                                              (128, 1))),
        "fc2W_hi": fc2_hi, "fc2W_lo": fc2_lo,
        "fc2bT": np.ascontiguousarray(f(inputs["fc2_b"])[:, None]),
        "embW_hi": emb_hi, "embW_lo": emb_lo,
        "embbT": np.ascontiguousarray(emb_b.reshape(KC_E, 128).T),
        "ident": np.eye(128, dtype=np.float32),
    }

    in_maps = []
    for core in range(NC):
        g, fr = core // NF, core % NF
        m = dict(base)
        colidx = np.concatenate([
            np.arange(gi * H + fr * HS, gi * H + (fr + 1) * HS)
            for gi in range(4)])
        for l in (0, 1):
            m[f"Wih{l}_hi"], m[f"Wih{l}_lo"] = _hilo(kmaj(Wg[l]["ih"][:, colidx]))
            m[f"Whh{l}_hi"], m[f"Whh{l}_lo"] = _hilo(kmaj(Wg[l]["hh"][:, colidx]))
            m[f"b{l}T"] = np.ascontiguousarray(
                Wg[l]["b"][colidx].reshape(8, 128).T)
        bsl = slice(g * BG, (g + 1) * BG)
        m["xT0_hi"], m["xT0_lo"] = _hilo(np.ascontiguousarray(x0[bsl].T))
        for l, hname in [(0, "h0T0"), (1, "h1T0")]:
            hT = np.ascontiguousarray(
                hn[l, bsl].T.reshape(KC_H, 128, BG).transpose(1, 0, 2))
            m[f"{hname}_hi"], m[f"{hname}_lo"] = _hilo(hT)
        for l, cname in [(0, "c00"), (1, "c10")]:
            cT = cn[l, bsl, fr * HS:(fr + 1) * HS].T  # [HS, BG]
            m[cname] = np.ascontiguousarray(
                cT.reshape(2, 128, BG).transpose(1, 0, 2).reshape(128, 2 * BG))
        in_maps.append(m)
    return in_maps


def kernel(**inputs):
    t_steps = int(inputs.get("forecast_window", T_FULL))
    nc, io = build(t_steps)
    in_maps = make_in_maps(inputs)
    r = bass_utils.run_bass_kernel_spmd(nc, in_maps, core_ids=list(range(NC)))
    out = np.empty((B, t_steps, D), np.float32)
    for g in range(NG):
        ysT = r.results[g * NF]["ysT"]          # [64, t, BG]
        out[g * BG:(g + 1) * BG] = ysT.transpose(2, 1, 0)
    return out
